# revision 18
# baseline (speedup 1.0000x reference)
"""DecoderRNN single-step decode on 8 Trainium2 NeuronCores.

Strategy (tensor-parallel, everything sharded):
  - Host gathers the embedding row (pure indexing) and pre-transposes /
    shards / bf16-casts all weights per core into DMA-friendly blocked
    layouts (contiguous per transfer, long rows).
  - Core k computes h_new[k*128:(k+1)*128] (GRU slices), its 256 rows of
    attention scores, a partial ctx over its encoder shard, its 128-slice
    of comb, and its ~6283-row shard of W_out logits.
  - Cross-core exchanges use 5 small collectives: AllGather(h_new),
    AllGather(scores), AllReduce(ctx), AllGather(comb),
    AllGather(log-softmax stats).
  - log_softmax: per-core max m_k / sum s_k = sum exp(l - m_k); global
    C = M + log(sum_k s_k * exp(m_k - M)); each core outputs l - C.
  - Matmul weights in bf16 (error ~4e-4 on log_probs); h_new update,
    softmax, biases, and all exchanged values stay fp32.

Self-contained: shapes hardcoded, no sibling imports.
"""

import os
import numpy as np

import concourse.bacc as bacc
import concourse.bass as bass
import concourse.tile as tile
from concourse.tile_rust import add_dep_helper
import concourse.mybir as mybir
from concourse.bass_utils import run_bass_kernel_spmd

F32 = mybir.dt.float32
BF16 = mybir.dt.bfloat16
AX = mybir.AxisListType.X
ALU = mybir.AluOpType
ACT = mybir.ActivationFunctionType

N_CORES = 8
H = 1024
V = 50257
S = 2048
HC = H // 128          # 8 h-chunks
SPC = S // N_CORES     # 256 encoder rows per core
VPC = 6400             # padded W_out rows per core (50 tiles of 128)
NVT = VPC // 128       # 50 v-tiles per core
BW = 640               # v-cols per psum bank pass (5 v-tiles)
NB = VPC // BW         # 10 v-blocks
NEG = -1.0e9           # pad bias so padded logits never matter

# packed fp32 const block column map
C_X8, C_HOWN, C_BRZ, C_BIN, C_BHN, C_BCB, C_ONE, C_BOUT, C_ID, C_TOT = (
    0, 8, 9, 11, 12, 13, 14, 15, 65, 193)

_ROWS = [6283] * 7 + [V - 7 * 6283]   # real W_out rows per core
_OFFS = np.cumsum([0] + _ROWS)

_NC_CACHE = {}


def _build_nc():
    if "nc" in _NC_CACHE:
        return _NC_CACHE["nc"]
    nc = bacc.Bacc("TRN2", target_bir_lowering=False, debug=False,
                   num_devices=N_CORES)
    rg = [list(range(N_CORES))]

    # ---- per-core inputs (blocked layouts, see _prep_inputs) -------------
    cpack_d = nc.dram_tensor("cpack", [128, C_TOT], F32, kind="ExternalInput")
    h8_d = nc.dram_tensor("h8", [128, HC], BF16, kind="ExternalInput")
    onesr_d = nc.dram_tensor("onesr", [1, 128], F32, kind="ExternalInput")
    wih_d = nc.dram_tensor("wihb", [4, 128, 768], BF16, kind="ExternalInput")
    whh_d = nc.dram_tensor("whhb", [4, 128, 768], BF16, kind="ExternalInput")
    encT_d = nc.dram_tensor("encTb", [4, 128, 512], BF16, kind="ExternalInput")
    encN_d = nc.dram_tensor("encNb", [2, 128, H], BF16, kind="ExternalInput")
    wcb_d = nc.dram_tensor("wcbb", [4, 128, 512], BF16, kind="ExternalInput")
    wo_d = nc.dram_tensor("wob", [2 * NB, 128, 4 * BW], BF16,
                          kind="ExternalInput")

    # ---- outputs ---------------------------------------------------------
    out_lg_d = nc.dram_tensor("out_logits", [128, NVT], F32,
                              kind="ExternalOutput")
    out_hn_d = nc.dram_tensor("out_hnew", [H], F32, kind="ExternalOutput")
    out_at_d = nc.dram_tensor("out_attn", [S], F32, kind="ExternalOutput")

    with tile.TileContext(nc) as tc:
        with (
            tc.tile_pool(name="w", bufs=1) as w,          # persistent weights
            tc.tile_pool(name="wo", bufs=20) as wo,        # W_out stream
            tc.tile_pool(name="sb", bufs=1) as sb,        # small working tiles
            tc.tile_pool(name="psA", bufs=2, space="PSUM") as psA,
            tc.tile_pool(name="psB", bufs=2, space="PSUM") as psB,
            tc.tile_pool(name="psW", bufs=3, space="PSUM") as psW,
            tc.tile_pool(name="psT", bufs=1, space="PSUM") as psT,
            tc.tile_pool(name="dr", bufs=1, space="DRAM") as dr,
            tc.tile_pool(name="drs", bufs=1, space="DRAM") as drs,
        ):
            # ---- front loads: GRU-critical first, on Vector's DGE --------
            cpack = w.tile([128, C_TOT], F32)
            nc.sync.dma_start(out=cpack[:], in_=cpack_d[:])
            h8 = w.tile([128, HC], BF16)
            nc.sync.dma_start(out=h8[:], in_=h8_d[:])
            onesr = w.tile([1, 128], F32)
            nc.sync.dma_start(out=onesr[:], in_=onesr_d[:])
            wihA, whhA = [], []
            for q in range(4):
                t1 = w.tile([128, 768], BF16, name=f"wih_{q}")
                nc.sync.dma_start(out=t1[:], in_=wih_d[q])
                wihA.append(t1)
                t2 = w.tile([128, 768], BF16, name=f"whh_{q}")
                nc.sync.dma_start(out=t2[:], in_=whh_d[q])
                whhA.append(t2)
            encTA = []
            for q in range(4):
                t = w.tile([128, 512], BF16, name=f"encT_{q}")
                nc.sync.dma_start(out=t[:], in_=encT_d[q])
                encTA.append(t)
            encNA = []
            for t_i in range(2):
                t = w.tile([128, H], BF16, name=f"encN_{t_i}")
                nc.sync.dma_start(out=t[:], in_=encN_d[t_i])
                encNA.append(t)
            wcbA = []
            for q in range(4):
                t = w.tile([128, 512], BF16, name=f"wcb_{q}")
                last_front = nc.sync.dma_start(out=t[:], in_=wcb_d[q])
                wcbA.append(t)

            x8 = cpack[:, C_X8:C_X8 + 8]
            hown = cpack[:, C_HOWN:C_HOWN + 1]
            brz = cpack[:, C_BRZ:C_BRZ + 2]
            bin_ = cpack[:, C_BIN:C_BIN + 1]
            bhn = cpack[:, C_BHN:C_BHN + 1]
            bcb = cpack[:, C_BCB:C_BCB + 1]
            ones = cpack[:, C_ONE:C_ONE + 1]
            bout_sb = cpack[:, C_BOUT:C_BOUT + NVT]
            ident = cpack[:, C_ID:C_ID + 128]

            def wih_l(c, g):
                return wihA[c // 2][:, (c % 2) * 384 + g * 128:
                                    (c % 2) * 384 + (g + 1) * 128]

            def whh_l(c, g):
                return whhA[c // 2][:, (c % 2) * 384 + g * 128:
                                    (c % 2) * 384 + (g + 1) * 128]

            def encT_l(c, t_i):
                return encTA[c // 2][:, (c % 2) * 256 + t_i * 128:
                                     (c % 2) * 256 + (t_i + 1) * 128]

            def encN_l(t_i, j):
                return encNA[t_i][:, j * 128:(j + 1) * 128]

            def wcb_l(c):
                return wcbA[c // 4][:, (c % 4) * 128:(c % 4 + 1) * 128]

            # ---- W_out stream DMAs on Sync's DGE (20 contiguous xfers) ---
            wo_half = []
            for i in range(2 * NB):
                t = wo.tile([128, 4 * BW], BF16, tag="wo", name=f"wo_{i}")
                wdma = nc.sync.dma_start(out=t[:], in_=wo_d[i])
                # keep the bulk W_out stream out of the front-loads' way
                add_dep_helper(wdma.ins, last_front.ins, sync=True,
                               reason="wo stream after front loads")
                wo_half.append(t)

            def wo_l(vb, c, vt):
                return wo_half[vb * 2 + c // 4][
                    :, (c % 4) * BW + vt * 128:(c % 4) * BW + (vt + 1) * 128]

            # ---- P1: GRU gates -------------------------------------------
            xr = sb.tile([128, HC], BF16)
            nc.scalar.activation(xr[:], x8, ACT.Relu)
            gi = psA.tile([128, 3], F32, tag="gru")
            gh = psA.tile([128, 3], F32, tag="gru")
            for g in range(3):
                for c in range(HC):
                    nc.tensor.matmul(gi[:, g:g + 1], lhsT=wih_l(c, g),
                                     rhs=xr[:, c:c + 1],
                                     start=(c == 0), stop=(c == HC - 1))
            for g in range(3):
                for c in range(HC):
                    nc.tensor.matmul(gh[:, g:g + 1], lhsT=whh_l(c, g),
                                     rhs=h8[:, c:c + 1],
                                     start=(c == 0), stop=(c == HC - 1))
            gisb = sb.tile([128, 3], F32)
            nc.vector.tensor_copy(gisb[:], gi[:])
            rzp = sb.tile([128, 2], F32)
            nc.vector.tensor_add(rzp[:], gisb[:, 0:2], gh[:, 0:2])
            nc.vector.tensor_add(rzp[:], rzp[:], brz)
            rz = sb.tile([128, 2], F32)
            nc.scalar.activation(rz[:], rzp[:], ACT.Sigmoid)
            npre = sb.tile([128, 1], F32)
            nc.vector.tensor_add(npre[:], gisb[:, 2:3], bin_)
            hnp = sb.tile([128, 1], F32)
            nc.vector.tensor_add(hnp[:], gh[:, 2:3], bhn)
            rhn = sb.tile([128, 1], F32)
            nc.vector.tensor_mul(rhn[:], rz[:, 0:1], hnp[:])
            nc.vector.tensor_add(npre[:], npre[:], rhn[:])
            n_t = sb.tile([128, 1], F32)
            nc.scalar.activation(n_t[:], npre[:], ACT.Tanh)
            dmn = sb.tile([128, 1], F32)
            nc.vector.tensor_sub(dmn[:], hown, n_t[:])
            nc.vector.tensor_mul(dmn[:], rz[:, 1:2], dmn[:])
            hno = sb.tile([128, 1], F32)
            nc.vector.tensor_add(hno[:], n_t[:], dmn[:])

            # ---- P2: AllGather h_new -------------------------------------
            hn_b = dr.tile([128], F32)
            nc.gpsimd.dma_start(out=hn_b[:].rearrange("(p f) -> p f", f=1),
                                in_=hno[:])
            hn_g = drs.tile([H], F32, addr_space="Shared")
            nc.gpsimd.collective_compute("AllGather", ALU.bypass,
                                         replica_groups=rg,
                                         ins=[hn_b[:]], outs=[hn_g[:]])
            nc.sync.dma_start(out=out_hn_d[:], in_=hn_g[:])
            hnew8 = sb.tile([128, HC], F32)
            nc.gpsimd.dma_start(out=hnew8[:],
                                in_=hn_g[:].rearrange("(f p) -> p f", p=128))
            hnew8b = sb.tile([128, HC], BF16)
            nc.vector.tensor_copy(hnew8b[:], hnew8[:])

            # ---- P3: attention scores ------------------------------------
            sc = psB.tile([128, 2], F32, tag="att")
            for t_i in range(2):
                for c in range(HC):
                    nc.tensor.matmul(sc[:, t_i:t_i + 1], lhsT=encT_l(c, t_i),
                                     rhs=hnew8b[:, c:c + 1],
                                     start=(c == 0), stop=(c == HC - 1))
            sc_sb = sb.tile([128, 2], F32)
            nc.vector.tensor_copy(sc_sb[:], sc[:])
            sc_b = dr.tile([SPC], F32)
            nc.gpsimd.dma_start(out=sc_b[:].rearrange("(f p) -> p f", p=128),
                                in_=sc_sb[:])
            sc_g = drs.tile([S], F32, addr_space="Shared")
            nc.gpsimd.collective_compute("AllGather", ALU.bypass,
                                         replica_groups=rg,
                                         ins=[sc_b[:]], outs=[sc_g[:]])

            # ---- P4: softmax ---------------------------------------------
            scf = sb.tile([128, 16], F32)
            nc.gpsimd.dma_start(out=scf[:],
                                in_=sc_g[:].rearrange("(p f) -> p f", f=16))
            m1 = sb.tile([128, 1], F32)
            nc.vector.tensor_reduce(m1[:], scf[:], axis=AX, op=ALU.max)
            m1t = psT.tile([1, 128], F32, tag="tp", name="m1t")
            nc.tensor.transpose(m1t[:], m1[:], ident)
            mx = sb.tile([1, 1], F32)
            nc.vector.tensor_reduce(mx[:], m1t[:], axis=AX, op=ALU.max)
            nmx = sb.tile([1, 1], F32)
            nc.scalar.mul(nmx[:], mx[:], -1.0)
            nmb_p = psT.tile([128, 1], F32, tag="tp", name="nmb_p")
            nc.tensor.matmul(nmb_p[:], lhsT=onesr[:], rhs=nmx[:],
                             start=True, stop=True)
            nmb = sb.tile([128, 1], F32)
            nc.vector.tensor_copy(nmb[:], nmb_p[:])
            ex = sb.tile([128, 16], F32)
            sums = sb.tile([128, 1], F32)
            nc.scalar.activation(ex[:], scf[:], ACT.Exp, bias=nmb[:],
                                 accum_out=sums[:])
            z_p = psT.tile([1, 1], F32, tag="tp", name="z_p")
            nc.tensor.matmul(z_p[:], lhsT=sums[:], rhs=ones,
                             start=True, stop=True)
            invz = sb.tile([1, 1], F32)
            nc.vector.reciprocal(invz[:], z_p[:])
            izb_p = psT.tile([128, 1], F32, tag="tp", name="izb_p")
            nc.tensor.matmul(izb_p[:], lhsT=onesr[:], rhs=invz[:],
                             start=True, stop=True)
            izb = sb.tile([128, 1], F32)
            nc.vector.tensor_copy(izb[:], izb_p[:])
            attn = sb.tile([128, 16], F32)
            nc.vector.tensor_scalar_mul(attn[:], ex[:], izb[:])
            nc.sync.dma_start(out=out_at_d[:].rearrange("(p f) -> p f", f=16),
                              in_=attn[:])
            el = sb.tile([128, 2], F32)
            nc.scalar.activation(el[:], sc_sb[:], ACT.Exp, bias=nmb[:])
            al = sb.tile([128, 2], BF16)
            nc.vector.tensor_scalar_mul(al[:], el[:], izb[:])

            # ---- P5: partial ctx over own encoder shard ------------------
            ctxp = psB.tile([128, HC], F32, tag="att")
            for j in range(HC):
                for t_i in range(2):
                    nc.tensor.matmul(ctxp[:, j:j + 1], lhsT=encN_l(t_i, j),
                                     rhs=al[:, t_i:t_i + 1],
                                     start=(t_i == 0), stop=(t_i == 1))
            ctx_sb = sb.tile([128, HC], F32)
            nc.vector.tensor_copy(ctx_sb[:], ctxp[:])
            ctx_b = dr.tile([H], F32)
            nc.gpsimd.dma_start(out=ctx_b[:].rearrange("(p f) -> p f", f=HC),
                                in_=ctx_sb[:])
            ctx_g = drs.tile([H], F32, addr_space="Shared", name="ctx_g")
            nc.gpsimd.collective_compute("AllReduce", ALU.add,
                                         replica_groups=rg,
                                         ins=[ctx_b[:]], outs=[ctx_g[:]])
            ctx8 = sb.tile([128, HC], F32)
            nc.gpsimd.dma_start(out=ctx8[:],
                                in_=ctx_g[:].rearrange("(p f) -> p f", f=HC))
            ctx8b = sb.tile([128, HC], BF16)
            nc.vector.tensor_copy(ctx8b[:], ctx8[:])

            # ---- P6: comb slice ------------------------------------------
            cbp = psB.tile([128, 1], F32, tag="att")
            for c in range(16):
                rhs = (ctx8b[:, c:c + 1] if c < HC
                       else hnew8b[:, c - HC:c - HC + 1])
                nc.tensor.matmul(cbp[:], lhsT=wcb_l(c), rhs=rhs,
                                 start=(c == 0), stop=(c == 15))
            cb = sb.tile([128, 1], F32)
            nc.scalar.activation(cb[:], cbp[:], ACT.Tanh, bias=bcb)
            cb_b = dr.tile([128], F32)
            nc.gpsimd.dma_start(out=cb_b[:].rearrange("(p f) -> p f", f=1),
                                in_=cb[:])
            cb_g = drs.tile([H], F32, addr_space="Shared", name="cb_g")
            nc.gpsimd.collective_compute("AllGather", ALU.bypass,
                                         replica_groups=rg,
                                         ins=[cb_b[:]], outs=[cb_g[:]])
            comb8 = sb.tile([128, HC], F32)
            nc.gpsimd.dma_start(out=comb8[:],
                                in_=cb_g[:].rearrange("(f p) -> p f", p=128))
            comb8b = sb.tile([128, HC], BF16)
            nc.vector.tensor_copy(comb8b[:], comb8[:])

            # ---- P7: W_out matvec + streaming stats ----------------------
            logits = sb.tile([128, NVT], F32)
            rmax = sb.tile([128, 1], F32)
            nc.vector.memset(rmax[:], NEG)
            nvb = BW // 128
            for vb in range(NB):
                ps = psW.tile([128, nvb], F32, tag="wops", name=f"wops_{vb}")
                for vt in range(nvb):
                    for c in range(HC):
                        nc.tensor.matmul(ps[:, vt:vt + 1],
                                         lhsT=wo_l(vb, c, vt),
                                         rhs=comb8b[:, c:c + 1],
                                         start=(c == 0), stop=(c == HC - 1))
                lsl = logits[:, vb * nvb:(vb + 1) * nvb]
                nc.vector.tensor_add(lsl, ps[:],
                                     bout_sb[:, vb * nvb:(vb + 1) * nvb])
                bm = sb.tile([128, 1], F32, tag="bm", name=f"bm_{vb}")
                nc.vector.tensor_reduce(bm[:], lsl, axis=AX, op=ALU.max)
                nc.vector.tensor_max(rmax[:], rmax[:], bm[:])

            # ---- P8: log-softmax stats + final ---------------------------
            rmt = psT.tile([1, 128], F32, tag="tp", name="rmt")
            nc.tensor.transpose(rmt[:], rmax[:], ident)
            mk = sb.tile([1, 1], F32)
            nc.vector.tensor_reduce(mk[:], rmt[:], axis=AX, op=ALU.max)
            nmk = sb.tile([1, 1], F32)
            nc.scalar.mul(nmk[:], mk[:], -1.0)
            nmkb_p = psT.tile([128, 1], F32, tag="tp", name="nmkb_p")
            nc.tensor.matmul(nmkb_p[:], lhsT=onesr[:], rhs=nmk[:],
                             start=True, stop=True)
            nmkb = sb.tile([128, 1], F32)
            nc.vector.tensor_copy(nmkb[:], nmkb_p[:])
            esc = sb.tile([128, NVT], F32)
            sumk = sb.tile([128, 1], F32)
            nc.scalar.activation(esc[:], logits[:], ACT.Exp, bias=nmkb[:],
                                 accum_out=sumk[:])
            sk_p = psT.tile([1, 1], F32, tag="tp", name="sk_p")
            nc.tensor.matmul(sk_p[:], lhsT=sumk[:], rhs=ones,
                             start=True, stop=True)
            st2 = sb.tile([1, 2], F32)
            nc.vector.tensor_copy(st2[0:1, 0:1], mk[:])
            nc.vector.tensor_copy(st2[0:1, 1:2], sk_p[:])
            st_b = dr.tile([2], F32)
            nc.gpsimd.dma_start(out=st_b[:].rearrange("(p f) -> p f", p=1),
                                in_=st2[:])
            st_g = drs.tile([2 * N_CORES], F32, addr_space="Shared",
                            name="st_g")
            nc.gpsimd.collective_compute("AllGather", ALU.bypass,
                                         replica_groups=rg,
                                         ins=[st_b[:]], outs=[st_g[:]])
            sg = sb.tile([1, 16], F32)
            nc.gpsimd.dma_start(out=sg[:],
                                in_=st_g[:].rearrange("(p f) -> p f", p=1))
            sgv = sg[:].rearrange("p (e two) -> p two e", two=2)
            mview = sgv[:, 0:1, :].rearrange("p a e -> p (a e)")
            sview = sgv[:, 1:2, :].rearrange("p a e -> p (a e)")
            gm = sb.tile([1, 1], F32)
            nc.vector.tensor_reduce(gm[:], mview, axis=AX, op=ALU.max)
            ngm = sb.tile([1, 1], F32)
            nc.scalar.mul(ngm[:], gm[:], -1.0)
            em8 = sb.tile([1, 8], F32)
            nc.scalar.activation(em8[:], mview, ACT.Exp, bias=ngm[:])
            zt8 = sb.tile([1, 8], F32)
            nc.vector.tensor_mul(zt8[:], em8[:], sview)
            zz = sb.tile([1, 1], F32)
            nc.vector.tensor_reduce(zz[:], zt8[:], axis=AX, op=ALU.add)
            lz = sb.tile([1, 1], F32)
            nc.scalar.activation(lz[:], zz[:], ACT.Ln)
            cc = sb.tile([1, 1], F32)
            nc.vector.tensor_add(cc[:], gm[:], lz[:])
            ncc = sb.tile([1, 1], F32)
            nc.scalar.mul(ncc[:], cc[:], -1.0)
            nccb_p = psT.tile([128, 1], F32, tag="tp", name="nccb_p")
            nc.tensor.matmul(nccb_p[:], lhsT=onesr[:], rhs=ncc[:],
                             start=True, stop=True)
            nccb = sb.tile([128, 1], F32)
            nc.vector.tensor_copy(nccb[:], nccb_p[:])
            outf = sb.tile([128, NVT], F32)
            nc.vector.tensor_scalar_add(outf[:], logits[:], nccb[:])
            nc.sync.dma_start(out=out_lg_d[:], in_=outf[:])

    nc.compile()
    _NC_CACHE["nc"] = nc
    return nc


def _prep_inputs(input_ids, hidden, encoder_outputs, emb_table, W_ih, W_hh,
                 b_ih, b_hh, W_comb, b_comb, W_out, b_out):
    """Shard + pre-layout all inputs per core (host-side, numpy)."""
    import ml_dtypes
    f = np.float32
    bf = ml_dtypes.bfloat16
    x_row = np.ascontiguousarray(emb_table[int(input_ids[0])], dtype=f)
    h_row = np.ascontiguousarray(hidden.reshape(H), dtype=f)
    x8 = np.ascontiguousarray(x_row.reshape(HC, 128).T)
    h8 = np.ascontiguousarray(h_row.reshape(HC, 128).T.astype(bf))
    onesr = np.ones((1, 128), dtype=f)
    b_ih = np.asarray(b_ih, dtype=f)
    b_hh = np.asarray(b_hh, dtype=f)
    bsum = b_ih + b_hh

    enc = np.asarray(encoder_outputs, dtype=f)
    encT_full = np.ascontiguousarray(enc.T)               # (H, S)
    W_ih = np.asarray(W_ih, dtype=f)
    W_hh = np.asarray(W_hh, dtype=f)
    W_comb = np.asarray(W_comb, dtype=f)
    W_out = np.asarray(W_out, dtype=f)
    b_out = np.asarray(b_out, dtype=f)
    b_comb = np.asarray(b_comb, dtype=f)

    in_maps = []
    for k in range(N_CORES):
        sl = slice(k * 128, (k + 1) * 128)
        # gate-sliced transposed GRU weights: (H, 384) cols = [r|z|n],
        # packed into quarters of 2 h-chunks: (4, 128, 2*384)
        wihT = np.empty((H, 384), dtype=bf)
        whhT = np.empty((H, 384), dtype=bf)
        for g in range(3):
            wihT[:, g * 128:(g + 1) * 128] = W_ih[g * H + k * 128:
                                                  g * H + (k + 1) * 128, :].T
            whhT[:, g * 128:(g + 1) * 128] = W_hh[g * H + k * 128:
                                                  g * H + (k + 1) * 128, :].T
        wihb = np.ascontiguousarray(
            wihT.reshape(4, 2, 128, 384).transpose(0, 2, 1, 3).reshape(
                4, 128, 768))
        whhb = np.ascontiguousarray(
            whhT.reshape(4, 2, 128, 384).transpose(0, 2, 1, 3).reshape(
                4, 128, 768))
        encT = encT_full[:, k * SPC:(k + 1) * SPC].astype(bf)  # (H, 256)
        encTb = np.ascontiguousarray(
            encT.reshape(4, 2, 128, 256).transpose(0, 2, 1, 3).reshape(
                4, 128, 512))
        encNb = np.ascontiguousarray(
            enc[k * SPC:(k + 1) * SPC, :].astype(bf).reshape(2, 128, H))
        wcbT = W_comb[sl, :].T.astype(bf)                  # (2H, 128)
        wcbb = np.ascontiguousarray(
            wcbT.reshape(4, 4, 128, 128).transpose(0, 2, 1, 3).reshape(
                4, 128, 512))
        brz = np.stack([bsum[0 * H + k * 128:0 * H + (k + 1) * 128],
                        bsum[1 * H + k * 128:1 * H + (k + 1) * 128]], axis=1)
        lo, hi = int(_OFFS[k]), int(_OFFS[k + 1])
        r = hi - lo
        woutT = np.zeros((H, VPC), dtype=bf)
        woutT[:, :r] = W_out[lo:hi, :].T.astype(bf)
        # (2*NB, 128, 4*BW): half-block i = (vb, h) covers chunks 4h..4h+3,
        # contiguous per partition row
        wob = np.ascontiguousarray(
            woutT.reshape(HC, 128, NB, BW)      # (c, p, vb, v)
            .transpose(2, 0, 1, 3)              # (vb, c, p, v)
            .reshape(NB, 2, 4, 128, BW)         # (vb, h, c4, p, v)
            .transpose(0, 1, 3, 2, 4)           # (vb, h, p, c4, v)
            .reshape(2 * NB, 128, 4 * BW))
        b_pad = np.full(VPC, NEG, dtype=f)
        b_pad[:r] = b_out[lo:hi]
        cpack = np.zeros((128, C_TOT), dtype=f)
        cpack[:, C_X8:C_X8 + 8] = x8
        cpack[:, C_HOWN] = h_row[sl]
        cpack[:, C_BRZ:C_BRZ + 2] = brz
        cpack[:, C_BIN] = b_ih[2 * H + k * 128:2 * H + (k + 1) * 128]
        cpack[:, C_BHN] = b_hh[2 * H + k * 128:2 * H + (k + 1) * 128]
        cpack[:, C_BCB] = b_comb[sl]
        cpack[:, C_ONE] = 1.0
        cpack[:, C_BOUT:C_BOUT + NVT] = b_pad.reshape(NVT, 128).T
        cpack[:, C_ID:C_ID + 128] = np.eye(128, dtype=f)
        in_maps.append({
            "cpack": cpack, "h8": h8, "onesr": onesr,
            "wihb": wihb, "whhb": whhb, "encTb": encTb, "encNb": encNb,
            "wcbb": wcbb, "wob": wob,
        })
    return in_maps


def _assemble(results):
    log_probs = np.empty((1, V), dtype=np.float32)
    for k in range(N_CORES):
        lg = np.asarray(results[k]["out_logits"]).reshape(128, NVT)
        shard = lg.T.reshape(VPC)
        lo, hi = int(_OFFS[k]), int(_OFFS[k + 1])
        log_probs[0, lo:hi] = shard[:hi - lo]
    h_new = np.asarray(results[0]["out_hnew"]).reshape(1, 1, H)
    attn = np.asarray(results[0]["out_attn"]).reshape(S)
    return log_probs, h_new, attn


_LAST_EXEC_NS = {"ns": None}


def kernel(**inputs):
    nc = _build_nc()
    in_maps = _prep_inputs(**inputs)
    if os.environ.get("KERNEL_SIM"):
        from concourse.bass_interp import MultiCoreSim
        sim = MultiCoreSim(nc, N_CORES)
        for i in range(N_CORES):
            for name, arr in in_maps[i].items():
                sim.cores[i].tensor(name)[:] = arr
        sim.simulate(check_with_hw=False)
        results = [{name: np.asarray(sim.cores[i].mem_tensor(name))
                    for name in ("out_logits", "out_hnew", "out_attn")}
                   for i in range(N_CORES)]
    else:
        trace = bool(os.environ.get("KERNEL_TRACE"))
        res = run_bass_kernel_spmd(nc, in_maps, list(range(N_CORES)),
                                   trace=trace)
        _LAST_EXEC_NS["ns"] = res.exec_time_ns
        results = res.results
    return _assemble(results)


# revision 19
# speedup vs baseline: 1.1328x; 1.1328x over previous
"""DecoderRNN single-step decode on 8 Trainium2 NeuronCores.

Strategy (tensor-parallel, everything sharded):
  - Host gathers the embedding row (pure indexing) and pre-transposes /
    shards / bf16-casts all weights per core into DMA-friendly blocked
    layouts (contiguous per transfer, long rows).
  - Core k computes h_new[k*128:(k+1)*128] (GRU slices), its 256 rows of
    attention scores, a partial ctx over its encoder shard, its 128-slice
    of comb, and its ~6283-row shard of W_out logits.
  - Cross-core exchanges use 5 small collectives: AllGather(h_new),
    AllGather(scores), AllReduce(ctx), AllGather(comb),
    AllGather(log-softmax stats).
  - log_softmax: per-core max m_k / sum s_k = sum exp(l - m_k); global
    C = M + log(sum_k s_k * exp(m_k - M)); each core outputs l - C.
  - Matmul weights in bf16 (error ~4e-4 on log_probs); h_new update,
    softmax, biases, and all exchanged values stay fp32.

Self-contained: shapes hardcoded, no sibling imports.
"""

import os
import numpy as np

import concourse.bacc as bacc
import concourse.bass as bass
import concourse.tile as tile
from concourse.tile_rust import add_dep_helper
import concourse.mybir as mybir
from concourse.bass_utils import run_bass_kernel_spmd

F32 = mybir.dt.float32
BF16 = mybir.dt.bfloat16
AX = mybir.AxisListType.X
ALU = mybir.AluOpType
ACT = mybir.ActivationFunctionType

N_CORES = 8
H = 1024
V = 50257
S = 2048
HC = H // 128          # 8 h-chunks
SPC = S // N_CORES     # 256 encoder rows per core
VPC = 6400             # padded W_out rows per core (50 tiles of 128)
NVT = VPC // 128       # 50 v-tiles per core
BW = 640               # v-cols per psum bank pass (5 v-tiles)
NB = VPC // BW         # 10 v-blocks
NEG = -1.0e9           # pad bias so padded logits never matter

# packed fp32 const block column map
C_X8, C_HOWN, C_BRZ, C_BIN, C_BHN, C_BCB, C_ONE, C_BOUT, C_ID, C_TOT = (
    0, 8, 9, 11, 12, 13, 14, 15, 65, 193)

_ROWS = [6283] * 7 + [V - 7 * 6283]   # real W_out rows per core
_OFFS = np.cumsum([0] + _ROWS)

_NC_CACHE = {}


def _build_nc():
    if "nc" in _NC_CACHE:
        return _NC_CACHE["nc"]
    nc = bacc.Bacc("TRN2", target_bir_lowering=False, debug=False,
                   num_devices=N_CORES)
    rg = [list(range(N_CORES))]

    # ---- per-core inputs (blocked layouts, see _prep_inputs) -------------
    cpack_d = nc.dram_tensor("cpack", [128, C_TOT], F32, kind="ExternalInput")
    h8_d = nc.dram_tensor("h8", [128, HC], BF16, kind="ExternalInput")
    onesr_d = nc.dram_tensor("onesr", [1, 128], F32, kind="ExternalInput")
    wih_d = nc.dram_tensor("wihb", [4, 128, 768], BF16, kind="ExternalInput")
    whh_d = nc.dram_tensor("whhb", [4, 128, 768], BF16, kind="ExternalInput")
    encT_d = nc.dram_tensor("encTb", [4, 128, 512], BF16, kind="ExternalInput")
    encN_d = nc.dram_tensor("encNb", [2, 128, H], BF16, kind="ExternalInput")
    wcb_d = nc.dram_tensor("wcbb", [4, 128, 512], BF16, kind="ExternalInput")
    wo_d = nc.dram_tensor("wob", [2 * NB, 128, 4 * BW], BF16,
                          kind="ExternalInput")

    # ---- outputs ---------------------------------------------------------
    out_lg_d = nc.dram_tensor("out_logits", [128, NVT], F32,
                              kind="ExternalOutput")
    out_hn_d = nc.dram_tensor("out_hnew", [H], F32, kind="ExternalOutput")
    out_at_d = nc.dram_tensor("out_attn", [S], F32, kind="ExternalOutput")

    with tile.TileContext(nc) as tc:
        with (
            tc.tile_pool(name="w", bufs=1) as w,          # persistent weights
            tc.tile_pool(name="wo", bufs=20) as wo,        # W_out stream
            tc.tile_pool(name="sb", bufs=1) as sb,        # small working tiles
            tc.tile_pool(name="psA", bufs=2, space="PSUM") as psA,
            tc.tile_pool(name="psB", bufs=2, space="PSUM") as psB,
            tc.tile_pool(name="psW", bufs=3, space="PSUM") as psW,
            tc.tile_pool(name="psT", bufs=1, space="PSUM") as psT,
            tc.tile_pool(name="dr", bufs=1, space="DRAM") as dr,
            tc.tile_pool(name="drs", bufs=1, space="DRAM") as drs,
        ):
            # ---- front loads: GRU-critical first, on Vector's DGE --------
            cpack = w.tile([128, C_TOT], F32)
            nc.sync.dma_start(out=cpack[:], in_=cpack_d[:])
            h8 = w.tile([128, HC], BF16)
            nc.sync.dma_start(out=h8[:], in_=h8_d[:])
            onesr = w.tile([1, 128], F32)
            nc.sync.dma_start(out=onesr[:], in_=onesr_d[:])
            wihA, whhA = [], []
            for q in range(4):
                t1 = w.tile([128, 768], BF16, name=f"wih_{q}")
                nc.sync.dma_start(out=t1[:], in_=wih_d[q])
                wihA.append(t1)
                t2 = w.tile([128, 768], BF16, name=f"whh_{q}")
                nc.sync.dma_start(out=t2[:], in_=whh_d[q])
                whhA.append(t2)
            encTA = []
            for q in range(4):
                t = w.tile([128, 512], BF16, name=f"encT_{q}")
                nc.sync.dma_start(out=t[:], in_=encT_d[q])
                encTA.append(t)
            encNA = []
            for t_i in range(2):
                t = w.tile([128, H], BF16, name=f"encN_{t_i}")
                nc.sync.dma_start(out=t[:], in_=encN_d[t_i])
                encNA.append(t)
            wcbA = []
            for q in range(4):
                t = w.tile([128, 512], BF16, name=f"wcb_{q}")
                last_front = nc.sync.dma_start(out=t[:], in_=wcb_d[q])
                wcbA.append(t)

            x8 = cpack[:, C_X8:C_X8 + 8]
            hown = cpack[:, C_HOWN:C_HOWN + 1]
            brz = cpack[:, C_BRZ:C_BRZ + 2]
            bin_ = cpack[:, C_BIN:C_BIN + 1]
            bhn = cpack[:, C_BHN:C_BHN + 1]
            bcb = cpack[:, C_BCB:C_BCB + 1]
            ones = cpack[:, C_ONE:C_ONE + 1]
            bout_sb = cpack[:, C_BOUT:C_BOUT + NVT]
            ident = cpack[:, C_ID:C_ID + 128]

            def wih_l(c, g):
                return wihA[c // 2][:, (c % 2) * 384 + g * 128:
                                    (c % 2) * 384 + (g + 1) * 128]

            def whh_l(c, g):
                return whhA[c // 2][:, (c % 2) * 384 + g * 128:
                                    (c % 2) * 384 + (g + 1) * 128]

            def encT_l(c, t_i):
                return encTA[c // 2][:, (c % 2) * 256 + t_i * 128:
                                     (c % 2) * 256 + (t_i + 1) * 128]

            def encN_l(t_i, j):
                return encNA[t_i][:, j * 128:(j + 1) * 128]

            def wcb_l(c):
                return wcbA[c // 4][:, (c % 4) * 128:(c % 4 + 1) * 128]

            # ---- W_out stream DMAs on Sync's DGE (20 contiguous xfers) ---
            wo_half = []
            for i in range(2 * NB):
                t = wo.tile([128, 4 * BW], BF16, tag="wo", name=f"wo_{i}")
                for hh in range(2):
                    wdma = nc.sync.dma_start(
                        out=t[:, hh * 2 * BW:(hh + 1) * 2 * BW],
                        in_=wo_d[i][:, hh * 2 * BW:(hh + 1) * 2 * BW])
                    # keep the bulk W_out stream out of the front-loads' way
                    add_dep_helper(wdma.ins, last_front.ins, sync=True,
                                   reason="wo stream after front loads")
                wo_half.append(t)

            def wo_l(vb, c, vt):
                return wo_half[vb * 2 + c // 4][
                    :, (c % 4) * BW + vt * 128:(c % 4) * BW + (vt + 1) * 128]

            # ---- P1: GRU gates -------------------------------------------
            xr = sb.tile([128, HC], BF16)
            nc.scalar.activation(xr[:], x8, ACT.Relu)
            gi = psA.tile([128, 3], F32, tag="gru")
            gh = psA.tile([128, 3], F32, tag="gru")
            for g in range(3):
                for c in range(HC):
                    nc.tensor.matmul(gi[:, g:g + 1], lhsT=wih_l(c, g),
                                     rhs=xr[:, c:c + 1],
                                     start=(c == 0), stop=(c == HC - 1))
            for g in range(3):
                for c in range(HC):
                    nc.tensor.matmul(gh[:, g:g + 1], lhsT=whh_l(c, g),
                                     rhs=h8[:, c:c + 1],
                                     start=(c == 0), stop=(c == HC - 1))
            gisb = sb.tile([128, 3], F32)
            nc.vector.tensor_copy(gisb[:], gi[:])
            rzp = sb.tile([128, 2], F32)
            nc.vector.tensor_add(rzp[:], gisb[:, 0:2], gh[:, 0:2])
            nc.vector.tensor_add(rzp[:], rzp[:], brz)
            rz = sb.tile([128, 2], F32)
            nc.scalar.activation(rz[:], rzp[:], ACT.Sigmoid)
            npre = sb.tile([128, 1], F32)
            nc.vector.tensor_add(npre[:], gisb[:, 2:3], bin_)
            hnp = sb.tile([128, 1], F32)
            nc.vector.tensor_add(hnp[:], gh[:, 2:3], bhn)
            rhn = sb.tile([128, 1], F32)
            nc.vector.tensor_mul(rhn[:], rz[:, 0:1], hnp[:])
            nc.vector.tensor_add(npre[:], npre[:], rhn[:])
            n_t = sb.tile([128, 1], F32)
            nc.scalar.activation(n_t[:], npre[:], ACT.Tanh)
            dmn = sb.tile([128, 1], F32)
            nc.vector.tensor_sub(dmn[:], hown, n_t[:])
            nc.vector.tensor_mul(dmn[:], rz[:, 1:2], dmn[:])
            hno = sb.tile([128, 1], F32)
            nc.vector.tensor_add(hno[:], n_t[:], dmn[:])

            # ---- P2: AllGather h_new -------------------------------------
            hn_b = dr.tile([128], F32)
            nc.scalar.dma_start(out=hn_b[:].rearrange("(p f) -> p f", f=1),
                                in_=hno[:])
            hn_g = drs.tile([H], F32, addr_space="Shared")
            nc.gpsimd.collective_compute("AllGather", ALU.bypass,
                                         replica_groups=rg,
                                         ins=[hn_b[:]], outs=[hn_g[:]])
            nc.sync.dma_start(out=out_hn_d[:], in_=hn_g[:])
            hnew8 = sb.tile([128, HC], F32)
            nc.scalar.dma_start(out=hnew8[:],
                                in_=hn_g[:].rearrange("(f p) -> p f", p=128))
            hnew8b = sb.tile([128, HC], BF16)
            nc.vector.tensor_copy(hnew8b[:], hnew8[:])

            # ---- P3: attention scores ------------------------------------
            sc = psB.tile([128, 2], F32, tag="att")
            for t_i in range(2):
                for c in range(HC):
                    nc.tensor.matmul(sc[:, t_i:t_i + 1], lhsT=encT_l(c, t_i),
                                     rhs=hnew8b[:, c:c + 1],
                                     start=(c == 0), stop=(c == HC - 1))
            sc_sb = sb.tile([128, 2], F32)
            nc.vector.tensor_copy(sc_sb[:], sc[:])
            sc_b = dr.tile([SPC], F32)
            nc.scalar.dma_start(out=sc_b[:].rearrange("(f p) -> p f", p=128),
                                in_=sc_sb[:])
            sc_g = drs.tile([S], F32, addr_space="Shared")
            nc.gpsimd.collective_compute("AllGather", ALU.bypass,
                                         replica_groups=rg,
                                         ins=[sc_b[:]], outs=[sc_g[:]])

            # ---- P4: softmax ---------------------------------------------
            scf = sb.tile([128, 16], F32)
            nc.scalar.dma_start(out=scf[:],
                                in_=sc_g[:].rearrange("(p f) -> p f", f=16))
            m1 = sb.tile([128, 1], F32)
            nc.vector.tensor_reduce(m1[:], scf[:], axis=AX, op=ALU.max)
            m1t = psT.tile([1, 128], F32, tag="tp", name="m1t")
            nc.tensor.transpose(m1t[:], m1[:], ident)
            mx = sb.tile([1, 1], F32)
            nc.vector.tensor_reduce(mx[:], m1t[:], axis=AX, op=ALU.max)
            nmx = sb.tile([1, 1], F32)
            nc.scalar.mul(nmx[:], mx[:], -1.0)
            nmb_p = psT.tile([128, 1], F32, tag="tp", name="nmb_p")
            nc.tensor.matmul(nmb_p[:], lhsT=onesr[:], rhs=nmx[:],
                             start=True, stop=True)
            nmb = sb.tile([128, 1], F32)
            nc.vector.tensor_copy(nmb[:], nmb_p[:])
            ex = sb.tile([128, 16], F32)
            sums = sb.tile([128, 1], F32)
            nc.scalar.activation(ex[:], scf[:], ACT.Exp, bias=nmb[:],
                                 accum_out=sums[:])
            z_p = psT.tile([1, 1], F32, tag="tp", name="z_p")
            nc.tensor.matmul(z_p[:], lhsT=sums[:], rhs=ones,
                             start=True, stop=True)
            invz = sb.tile([1, 1], F32)
            nc.vector.reciprocal(invz[:], z_p[:])
            izb_p = psT.tile([128, 1], F32, tag="tp", name="izb_p")
            nc.tensor.matmul(izb_p[:], lhsT=onesr[:], rhs=invz[:],
                             start=True, stop=True)
            izb = sb.tile([128, 1], F32)
            nc.vector.tensor_copy(izb[:], izb_p[:])
            attn = sb.tile([128, 16], F32)
            nc.vector.tensor_scalar_mul(attn[:], ex[:], izb[:])
            nc.sync.dma_start(out=out_at_d[:].rearrange("(p f) -> p f", f=16),
                              in_=attn[:])
            el = sb.tile([128, 2], F32)
            nc.scalar.activation(el[:], sc_sb[:], ACT.Exp, bias=nmb[:])
            al = sb.tile([128, 2], BF16)
            nc.vector.tensor_scalar_mul(al[:], el[:], izb[:])

            # ---- P5: partial ctx over own encoder shard ------------------
            ctxp = psB.tile([128, HC], F32, tag="att")
            for j in range(HC):
                for t_i in range(2):
                    nc.tensor.matmul(ctxp[:, j:j + 1], lhsT=encN_l(t_i, j),
                                     rhs=al[:, t_i:t_i + 1],
                                     start=(t_i == 0), stop=(t_i == 1))
            ctx_sb = sb.tile([128, HC], F32)
            nc.vector.tensor_copy(ctx_sb[:], ctxp[:])
            ctx_b = dr.tile([H], F32)
            nc.scalar.dma_start(out=ctx_b[:].rearrange("(p f) -> p f", f=HC),
                                in_=ctx_sb[:])
            ctx_g = drs.tile([H], F32, addr_space="Shared", name="ctx_g")
            nc.gpsimd.collective_compute("AllReduce", ALU.add,
                                         replica_groups=rg,
                                         ins=[ctx_b[:]], outs=[ctx_g[:]])
            ctx8 = sb.tile([128, HC], F32)
            nc.scalar.dma_start(out=ctx8[:],
                                in_=ctx_g[:].rearrange("(p f) -> p f", f=HC))
            ctx8b = sb.tile([128, HC], BF16)
            nc.vector.tensor_copy(ctx8b[:], ctx8[:])

            # ---- P6: comb slice ------------------------------------------
            cbp = psB.tile([128, 1], F32, tag="att")
            for c in range(16):
                rhs = (ctx8b[:, c:c + 1] if c < HC
                       else hnew8b[:, c - HC:c - HC + 1])
                nc.tensor.matmul(cbp[:], lhsT=wcb_l(c), rhs=rhs,
                                 start=(c == 0), stop=(c == 15))
            cb = sb.tile([128, 1], F32)
            nc.scalar.activation(cb[:], cbp[:], ACT.Tanh, bias=bcb)
            cb_b = dr.tile([128], F32)
            nc.scalar.dma_start(out=cb_b[:].rearrange("(p f) -> p f", f=1),
                                in_=cb[:])
            cb_g = drs.tile([H], F32, addr_space="Shared", name="cb_g")
            nc.gpsimd.collective_compute("AllGather", ALU.bypass,
                                         replica_groups=rg,
                                         ins=[cb_b[:]], outs=[cb_g[:]])
            comb8 = sb.tile([128, HC], F32)
            nc.scalar.dma_start(out=comb8[:],
                                in_=cb_g[:].rearrange("(f p) -> p f", p=128))
            comb8b = sb.tile([128, HC], BF16)
            nc.vector.tensor_copy(comb8b[:], comb8[:])

            # ---- P7: W_out matvec + streaming stats ----------------------
            logits = sb.tile([128, NVT], F32)
            rmax = sb.tile([128, 1], F32)
            nc.vector.memset(rmax[:], NEG)
            nvb = BW // 128
            for vb in range(NB):
                ps = psW.tile([128, nvb], F32, tag="wops", name=f"wops_{vb}")
                for vt in range(nvb):
                    for c in range(HC):
                        nc.tensor.matmul(ps[:, vt:vt + 1],
                                         lhsT=wo_l(vb, c, vt),
                                         rhs=comb8b[:, c:c + 1],
                                         start=(c == 0), stop=(c == HC - 1))
                lsl = logits[:, vb * nvb:(vb + 1) * nvb]
                nc.vector.tensor_add(lsl, ps[:],
                                     bout_sb[:, vb * nvb:(vb + 1) * nvb])
                bm = sb.tile([128, 1], F32, tag="bm", name=f"bm_{vb}")
                nc.vector.tensor_reduce(bm[:], lsl, axis=AX, op=ALU.max)
                nc.vector.tensor_max(rmax[:], rmax[:], bm[:])

            # ---- P8: log-softmax stats + final ---------------------------
            rmt = psT.tile([1, 128], F32, tag="tp", name="rmt")
            nc.tensor.transpose(rmt[:], rmax[:], ident)
            mk = sb.tile([1, 1], F32)
            nc.vector.tensor_reduce(mk[:], rmt[:], axis=AX, op=ALU.max)
            nmk = sb.tile([1, 1], F32)
            nc.scalar.mul(nmk[:], mk[:], -1.0)
            nmkb_p = psT.tile([128, 1], F32, tag="tp", name="nmkb_p")
            nc.tensor.matmul(nmkb_p[:], lhsT=onesr[:], rhs=nmk[:],
                             start=True, stop=True)
            nmkb = sb.tile([128, 1], F32)
            nc.vector.tensor_copy(nmkb[:], nmkb_p[:])
            esc = sb.tile([128, NVT], F32)
            sumk = sb.tile([128, 1], F32)
            nc.scalar.activation(esc[:], logits[:], ACT.Exp, bias=nmkb[:],
                                 accum_out=sumk[:])
            sk_p = psT.tile([1, 1], F32, tag="tp", name="sk_p")
            nc.tensor.matmul(sk_p[:], lhsT=sumk[:], rhs=ones,
                             start=True, stop=True)
            st2 = sb.tile([1, 2], F32)
            nc.vector.tensor_copy(st2[0:1, 0:1], mk[:])
            nc.vector.tensor_copy(st2[0:1, 1:2], sk_p[:])
            st_b = dr.tile([2], F32)
            nc.scalar.dma_start(out=st_b[:].rearrange("(p f) -> p f", p=1),
                                in_=st2[:])
            st_g = drs.tile([2 * N_CORES], F32, addr_space="Shared",
                            name="st_g")
            nc.gpsimd.collective_compute("AllGather", ALU.bypass,
                                         replica_groups=rg,
                                         ins=[st_b[:]], outs=[st_g[:]])
            sg = sb.tile([1, 16], F32)
            nc.scalar.dma_start(out=sg[:],
                                in_=st_g[:].rearrange("(p f) -> p f", p=1))
            sgv = sg[:].rearrange("p (e two) -> p two e", two=2)
            mview = sgv[:, 0:1, :].rearrange("p a e -> p (a e)")
            sview = sgv[:, 1:2, :].rearrange("p a e -> p (a e)")
            gm = sb.tile([1, 1], F32)
            nc.vector.tensor_reduce(gm[:], mview, axis=AX, op=ALU.max)
            ngm = sb.tile([1, 1], F32)
            nc.scalar.mul(ngm[:], gm[:], -1.0)
            em8 = sb.tile([1, 8], F32)
            nc.scalar.activation(em8[:], mview, ACT.Exp, bias=ngm[:])
            zt8 = sb.tile([1, 8], F32)
            nc.vector.tensor_mul(zt8[:], em8[:], sview)
            zz = sb.tile([1, 1], F32)
            nc.vector.tensor_reduce(zz[:], zt8[:], axis=AX, op=ALU.add)
            lz = sb.tile([1, 1], F32)
            nc.scalar.activation(lz[:], zz[:], ACT.Ln)
            cc = sb.tile([1, 1], F32)
            nc.vector.tensor_add(cc[:], gm[:], lz[:])
            ncc = sb.tile([1, 1], F32)
            nc.scalar.mul(ncc[:], cc[:], -1.0)
            nccb_p = psT.tile([128, 1], F32, tag="tp", name="nccb_p")
            nc.tensor.matmul(nccb_p[:], lhsT=onesr[:], rhs=ncc[:],
                             start=True, stop=True)
            nccb = sb.tile([128, 1], F32)
            nc.vector.tensor_copy(nccb[:], nccb_p[:])
            outf = sb.tile([128, NVT], F32)
            nc.vector.tensor_scalar_add(outf[:], logits[:], nccb[:])
            nc.sync.dma_start(out=out_lg_d[:], in_=outf[:])

    nc.compile()
    _NC_CACHE["nc"] = nc
    return nc


def _prep_inputs(input_ids, hidden, encoder_outputs, emb_table, W_ih, W_hh,
                 b_ih, b_hh, W_comb, b_comb, W_out, b_out):
    """Shard + pre-layout all inputs per core (host-side, numpy)."""
    import ml_dtypes
    f = np.float32
    bf = ml_dtypes.bfloat16
    x_row = np.ascontiguousarray(emb_table[int(input_ids[0])], dtype=f)
    h_row = np.ascontiguousarray(hidden.reshape(H), dtype=f)
    x8 = np.ascontiguousarray(x_row.reshape(HC, 128).T)
    h8 = np.ascontiguousarray(h_row.reshape(HC, 128).T.astype(bf))
    onesr = np.ones((1, 128), dtype=f)
    b_ih = np.asarray(b_ih, dtype=f)
    b_hh = np.asarray(b_hh, dtype=f)
    bsum = b_ih + b_hh

    enc = np.asarray(encoder_outputs, dtype=f)
    encT_full = np.ascontiguousarray(enc.T)               # (H, S)
    W_ih = np.asarray(W_ih, dtype=f)
    W_hh = np.asarray(W_hh, dtype=f)
    W_comb = np.asarray(W_comb, dtype=f)
    W_out = np.asarray(W_out, dtype=f)
    b_out = np.asarray(b_out, dtype=f)
    b_comb = np.asarray(b_comb, dtype=f)

    in_maps = []
    for k in range(N_CORES):
        sl = slice(k * 128, (k + 1) * 128)
        # gate-sliced transposed GRU weights: (H, 384) cols = [r|z|n],
        # packed into quarters of 2 h-chunks: (4, 128, 2*384)
        wihT = np.empty((H, 384), dtype=bf)
        whhT = np.empty((H, 384), dtype=bf)
        for g in range(3):
            wihT[:, g * 128:(g + 1) * 128] = W_ih[g * H + k * 128:
                                                  g * H + (k + 1) * 128, :].T
            whhT[:, g * 128:(g + 1) * 128] = W_hh[g * H + k * 128:
                                                  g * H + (k + 1) * 128, :].T
        wihb = np.ascontiguousarray(
            wihT.reshape(4, 2, 128, 384).transpose(0, 2, 1, 3).reshape(
                4, 128, 768))
        whhb = np.ascontiguousarray(
            whhT.reshape(4, 2, 128, 384).transpose(0, 2, 1, 3).reshape(
                4, 128, 768))
        encT = encT_full[:, k * SPC:(k + 1) * SPC].astype(bf)  # (H, 256)
        encTb = np.ascontiguousarray(
            encT.reshape(4, 2, 128, 256).transpose(0, 2, 1, 3).reshape(
                4, 128, 512))
        encNb = np.ascontiguousarray(
            enc[k * SPC:(k + 1) * SPC, :].astype(bf).reshape(2, 128, H))
        wcbT = W_comb[sl, :].T.astype(bf)                  # (2H, 128)
        wcbb = np.ascontiguousarray(
            wcbT.reshape(4, 4, 128, 128).transpose(0, 2, 1, 3).reshape(
                4, 128, 512))
        brz = np.stack([bsum[0 * H + k * 128:0 * H + (k + 1) * 128],
                        bsum[1 * H + k * 128:1 * H + (k + 1) * 128]], axis=1)
        lo, hi = int(_OFFS[k]), int(_OFFS[k + 1])
        r = hi - lo
        woutT = np.zeros((H, VPC), dtype=bf)
        woutT[:, :r] = W_out[lo:hi, :].T.astype(bf)
        # (2*NB, 128, 4*BW): half-block i = (vb, h) covers chunks 4h..4h+3,
        # contiguous per partition row
        wob = np.ascontiguousarray(
            woutT.reshape(HC, 128, NB, BW)      # (c, p, vb, v)
            .transpose(2, 0, 1, 3)              # (vb, c, p, v)
            .reshape(NB, 2, 4, 128, BW)         # (vb, h, c4, p, v)
            .transpose(0, 1, 3, 2, 4)           # (vb, h, p, c4, v)
            .reshape(2 * NB, 128, 4 * BW))
        b_pad = np.full(VPC, NEG, dtype=f)
        b_pad[:r] = b_out[lo:hi]
        cpack = np.zeros((128, C_TOT), dtype=f)
        cpack[:, C_X8:C_X8 + 8] = x8
        cpack[:, C_HOWN] = h_row[sl]
        cpack[:, C_BRZ:C_BRZ + 2] = brz
        cpack[:, C_BIN] = b_ih[2 * H + k * 128:2 * H + (k + 1) * 128]
        cpack[:, C_BHN] = b_hh[2 * H + k * 128:2 * H + (k + 1) * 128]
        cpack[:, C_BCB] = b_comb[sl]
        cpack[:, C_ONE] = 1.0
        cpack[:, C_BOUT:C_BOUT + NVT] = b_pad.reshape(NVT, 128).T
        cpack[:, C_ID:C_ID + 128] = np.eye(128, dtype=f)
        in_maps.append({
            "cpack": cpack, "h8": h8, "onesr": onesr,
            "wihb": wihb, "whhb": whhb, "encTb": encTb, "encNb": encNb,
            "wcbb": wcbb, "wob": wob,
        })
    return in_maps


def _assemble(results):
    log_probs = np.empty((1, V), dtype=np.float32)
    for k in range(N_CORES):
        lg = np.asarray(results[k]["out_logits"]).reshape(128, NVT)
        shard = lg.T.reshape(VPC)
        lo, hi = int(_OFFS[k]), int(_OFFS[k + 1])
        log_probs[0, lo:hi] = shard[:hi - lo]
    h_new = np.asarray(results[0]["out_hnew"]).reshape(1, 1, H)
    attn = np.asarray(results[0]["out_attn"]).reshape(S)
    return log_probs, h_new, attn


_LAST_EXEC_NS = {"ns": None}


def kernel(**inputs):
    nc = _build_nc()
    in_maps = _prep_inputs(**inputs)
    if os.environ.get("KERNEL_SIM"):
        from concourse.bass_interp import MultiCoreSim
        sim = MultiCoreSim(nc, N_CORES)
        for i in range(N_CORES):
            for name, arr in in_maps[i].items():
                sim.cores[i].tensor(name)[:] = arr
        sim.simulate(check_with_hw=False)
        results = [{name: np.asarray(sim.cores[i].mem_tensor(name))
                    for name in ("out_logits", "out_hnew", "out_attn")}
                   for i in range(N_CORES)]
    else:
        trace = bool(os.environ.get("KERNEL_TRACE"))
        res = run_bass_kernel_spmd(nc, in_maps, list(range(N_CORES)),
                                   trace=trace)
        _LAST_EXEC_NS["ns"] = res.exec_time_ns
        results = res.results
    return _assemble(results)


# revision 20
# speedup vs baseline: 1.2410x; 1.0955x over previous
"""DecoderRNN single-step decode on 8 Trainium2 NeuronCores.

Strategy (tensor-parallel, everything sharded):
  - Host gathers the embedding row (pure indexing) and pre-transposes /
    shards / bf16-casts all weights per core into DMA-friendly blocked
    layouts (contiguous per transfer, long rows).
  - Core k computes h_new[k*128:(k+1)*128] (GRU slices), its 256 rows of
    attention scores, a partial ctx over its encoder shard, its 128-slice
    of comb, and its ~6283-row shard of W_out logits.
  - Cross-core exchanges use 5 small collectives: AllGather(h_new),
    AllGather(scores), AllReduce(ctx), AllGather(comb),
    AllGather(log-softmax stats).
  - log_softmax: per-core max m_k / sum s_k = sum exp(l - m_k); global
    C = M + log(sum_k s_k * exp(m_k - M)); each core outputs l - C.
  - Matmul weights in bf16 (error ~4e-4 on log_probs); h_new update,
    softmax, biases, and all exchanged values stay fp32.

Self-contained: shapes hardcoded, no sibling imports.
"""

import os
import numpy as np

import concourse.bacc as bacc
import concourse.bass as bass
import concourse.tile as tile
from concourse.tile_rust import add_dep_helper
import concourse.mybir as mybir
from concourse.bass_utils import run_bass_kernel_spmd

F32 = mybir.dt.float32
BF16 = mybir.dt.bfloat16
AX = mybir.AxisListType.X
ALU = mybir.AluOpType
ACT = mybir.ActivationFunctionType

N_CORES = 8
H = 1024
V = 50257
S = 2048
HC = H // 128          # 8 h-chunks
SPC = S // N_CORES     # 256 encoder rows per core
VPC = 6400             # padded W_out rows per core (50 tiles of 128)
NVT = VPC // 128       # 50 v-tiles per core
BW = 640               # v-cols per psum bank pass (5 v-tiles)
NB = VPC // BW         # 10 v-blocks
NEG = -1.0e9           # pad bias so padded logits never matter

# packed fp32 const block column map
C_X8, C_HOWN, C_BRZ, C_BIN, C_BHN, C_BCB, C_ONE, C_BOUT, C_ID, C_TOT = (
    0, 8, 9, 11, 12, 13, 14, 15, 65, 193)

_ROWS = [6283] * 7 + [V - 7 * 6283]   # real W_out rows per core
_OFFS = np.cumsum([0] + _ROWS)

_NC_CACHE = {}


def _build_nc():
    if "nc" in _NC_CACHE:
        return _NC_CACHE["nc"]
    nc = bacc.Bacc("TRN2", target_bir_lowering=False, debug=False,
                   num_devices=N_CORES)
    rg = [list(range(N_CORES))]

    # ---- per-core inputs (blocked layouts, see _prep_inputs) -------------
    cpack_d = nc.dram_tensor("cpack", [128, C_TOT], F32, kind="ExternalInput")
    h8_d = nc.dram_tensor("h8", [128, HC], BF16, kind="ExternalInput")
    onesr_d = nc.dram_tensor("onesr", [1, 128], F32, kind="ExternalInput")
    wih_d = nc.dram_tensor("wihb", [4, 128, 768], BF16, kind="ExternalInput")
    whh_d = nc.dram_tensor("whhb", [4, 128, 768], BF16, kind="ExternalInput")
    encT_d = nc.dram_tensor("encTb", [4, 128, 512], BF16, kind="ExternalInput")
    encN_d = nc.dram_tensor("encNb", [2, 128, H], BF16, kind="ExternalInput")
    wcb_d = nc.dram_tensor("wcbb", [4, 128, 512], BF16, kind="ExternalInput")
    wo_d = nc.dram_tensor("wob", [2 * NB, 128, 4 * BW], BF16,
                          kind="ExternalInput")

    # ---- outputs ---------------------------------------------------------
    out_lg_d = nc.dram_tensor("out_logits", [128, NVT], F32,
                              kind="ExternalOutput")
    out_hn_d = nc.dram_tensor("out_hnew", [H], F32, kind="ExternalOutput")
    out_at_d = nc.dram_tensor("out_attn", [S], F32, kind="ExternalOutput")

    with tile.TileContext(nc) as tc:
        with (
            tc.tile_pool(name="w", bufs=1) as w,          # persistent weights
            tc.tile_pool(name="wo", bufs=20) as wo,        # W_out stream
            tc.tile_pool(name="sb", bufs=1) as sb,        # small working tiles
            tc.tile_pool(name="psA", bufs=2, space="PSUM") as psA,
            tc.tile_pool(name="psB", bufs=2, space="PSUM") as psB,
            tc.tile_pool(name="psW", bufs=3, space="PSUM") as psW,
            tc.tile_pool(name="psT", bufs=1, space="PSUM") as psT,
            tc.tile_pool(name="dr", bufs=1, space="DRAM") as dr,
            tc.tile_pool(name="drs", bufs=1, space="DRAM") as drs,
        ):
            # ---- front loads: GRU-critical first, on Vector's DGE --------
            cpack = w.tile([128, C_TOT], F32)
            nc.sync.dma_start(out=cpack[:], in_=cpack_d[:])
            h8 = w.tile([128, HC], BF16)
            nc.sync.dma_start(out=h8[:], in_=h8_d[:])
            onesr = w.tile([1, 128], F32)
            nc.sync.dma_start(out=onesr[:], in_=onesr_d[:])
            wihA, whhA = [], []
            for q in range(4):
                t1 = w.tile([128, 768], BF16, name=f"wih_{q}")
                nc.sync.dma_start(out=t1[:], in_=wih_d[q])
                wihA.append(t1)
                t2 = w.tile([128, 768], BF16, name=f"whh_{q}")
                nc.sync.dma_start(out=t2[:], in_=whh_d[q])
                whhA.append(t2)
            encTA = []
            for q in range(4):
                t = w.tile([128, 512], BF16, name=f"encT_{q}")
                nc.sync.dma_start(out=t[:], in_=encT_d[q])
                encTA.append(t)
            encNA = []
            for t_i in range(2):
                t = w.tile([128, H], BF16, name=f"encN_{t_i}")
                nc.sync.dma_start(out=t[:], in_=encN_d[t_i])
                encNA.append(t)
            wcbA = []
            for q in range(4):
                t = w.tile([128, 512], BF16, name=f"wcb_{q}")
                last_front = nc.sync.dma_start(out=t[:], in_=wcb_d[q])
                wcbA.append(t)

            x8 = cpack[:, C_X8:C_X8 + 8]
            hown = cpack[:, C_HOWN:C_HOWN + 1]
            brz = cpack[:, C_BRZ:C_BRZ + 2]
            bin_ = cpack[:, C_BIN:C_BIN + 1]
            bhn = cpack[:, C_BHN:C_BHN + 1]
            bcb = cpack[:, C_BCB:C_BCB + 1]
            ones = cpack[:, C_ONE:C_ONE + 1]
            bout_sb = cpack[:, C_BOUT:C_BOUT + NVT]
            ident = cpack[:, C_ID:C_ID + 128]

            def wih_l(c, g):
                return wihA[c // 2][:, (c % 2) * 384 + g * 128:
                                    (c % 2) * 384 + (g + 1) * 128]

            def whh_l(c, g):
                return whhA[c // 2][:, (c % 2) * 384 + g * 128:
                                    (c % 2) * 384 + (g + 1) * 128]

            def encT_l(c, t_i):
                return encTA[c // 2][:, (c % 2) * 256 + t_i * 128:
                                     (c % 2) * 256 + (t_i + 1) * 128]

            def encN_l(t_i, j):
                return encNA[t_i][:, j * 128:(j + 1) * 128]

            def wcb_l(c):
                return wcbA[c // 4][:, (c % 4) * 128:(c % 4 + 1) * 128]

            # ---- W_out stream DMAs on Sync's DGE (20 contiguous xfers) ---
            wo_half = []
            for i in range(2 * NB):
                t = wo.tile([128, 4 * BW], BF16, tag="wo", name=f"wo_{i}")
                wdma = nc.sync.dma_start(out=t[:], in_=wo_d[i])
                # keep the bulk W_out stream out of the front-loads' way
                add_dep_helper(wdma.ins, last_front.ins, sync=True,
                               reason="wo stream after front loads")
                wo_half.append(t)

            def wo_l(vb, c, vt):
                return wo_half[vb * 2 + c // 4][
                    :, (c % 4) * BW + vt * 128:(c % 4) * BW + (vt + 1) * 128]

            # ---- P1: GRU gates -------------------------------------------
            xr = sb.tile([128, HC], BF16)
            nc.scalar.activation(xr[:], x8, ACT.Relu)
            gi = psA.tile([128, 3], F32, tag="gru")
            gh = psA.tile([128, 3], F32, tag="gru")
            for g in range(3):
                for c in range(HC):
                    nc.tensor.matmul(gi[:, g:g + 1], lhsT=wih_l(c, g),
                                     rhs=xr[:, c:c + 1],
                                     start=(c == 0), stop=(c == HC - 1))
            for g in range(3):
                for c in range(HC):
                    nc.tensor.matmul(gh[:, g:g + 1], lhsT=whh_l(c, g),
                                     rhs=h8[:, c:c + 1],
                                     start=(c == 0), stop=(c == HC - 1))
            gisb = sb.tile([128, 3], F32)
            nc.vector.tensor_copy(gisb[:], gi[:])
            rzp = sb.tile([128, 2], F32)
            nc.vector.tensor_add(rzp[:], gisb[:, 0:2], gh[:, 0:2])
            nc.vector.tensor_add(rzp[:], rzp[:], brz)
            rz = sb.tile([128, 2], F32)
            nc.scalar.activation(rz[:], rzp[:], ACT.Sigmoid)
            npre = sb.tile([128, 1], F32)
            nc.vector.tensor_add(npre[:], gisb[:, 2:3], bin_)
            hnp = sb.tile([128, 1], F32)
            nc.vector.tensor_add(hnp[:], gh[:, 2:3], bhn)
            rhn = sb.tile([128, 1], F32)
            nc.vector.tensor_mul(rhn[:], rz[:, 0:1], hnp[:])
            nc.vector.tensor_add(npre[:], npre[:], rhn[:])
            n_t = sb.tile([128, 1], F32)
            nc.scalar.activation(n_t[:], npre[:], ACT.Tanh)
            dmn = sb.tile([128, 1], F32)
            nc.vector.tensor_sub(dmn[:], hown, n_t[:])
            nc.vector.tensor_mul(dmn[:], rz[:, 1:2], dmn[:])
            hno = sb.tile([128, 1], F32)
            nc.vector.tensor_add(hno[:], n_t[:], dmn[:])

            # ---- P2: AllGather h_new -------------------------------------
            hn_b = dr.tile([128], F32)
            nc.scalar.dma_start(out=hn_b[:].rearrange("(p f) -> p f", f=1),
                                in_=hno[:])
            hn_g = drs.tile([H], F32, addr_space="Shared")
            nc.gpsimd.collective_compute("AllGather", ALU.bypass,
                                         replica_groups=rg,
                                         ins=[hn_b[:]], outs=[hn_g[:]])
            nc.sync.dma_start(out=out_hn_d[:], in_=hn_g[:])
            hnew8 = sb.tile([128, HC], F32)
            nc.scalar.dma_start(out=hnew8[:],
                                in_=hn_g[:].rearrange("(f p) -> p f", p=128))
            hnew8b = sb.tile([128, HC], BF16)
            nc.vector.tensor_copy(hnew8b[:], hnew8[:])

            # ---- P3: attention scores ------------------------------------
            sc = psB.tile([128, 2], F32, tag="att")
            for t_i in range(2):
                for c in range(HC):
                    nc.tensor.matmul(sc[:, t_i:t_i + 1], lhsT=encT_l(c, t_i),
                                     rhs=hnew8b[:, c:c + 1],
                                     start=(c == 0), stop=(c == HC - 1))
            sc_sb = sb.tile([128, 2], F32)
            nc.vector.tensor_copy(sc_sb[:], sc[:])
            sc_b = dr.tile([SPC], F32)
            nc.scalar.dma_start(out=sc_b[:].rearrange("(f p) -> p f", p=128),
                                in_=sc_sb[:])
            sc_g = drs.tile([S], F32, addr_space="Shared")
            nc.gpsimd.collective_compute("AllGather", ALU.bypass,
                                         replica_groups=rg,
                                         ins=[sc_b[:]], outs=[sc_g[:]])

            # ---- P4: softmax ---------------------------------------------
            scf = sb.tile([128, 16], F32)
            nc.scalar.dma_start(out=scf[:],
                                in_=sc_g[:].rearrange("(p f) -> p f", f=16))
            m1 = sb.tile([128, 1], F32)
            nc.vector.tensor_reduce(m1[:], scf[:], axis=AX, op=ALU.max)
            m1t = psT.tile([1, 128], F32, tag="tp", name="m1t")
            nc.tensor.transpose(m1t[:], m1[:], ident)
            mx = sb.tile([1, 1], F32)
            nc.vector.tensor_reduce(mx[:], m1t[:], axis=AX, op=ALU.max)
            nmx = sb.tile([1, 1], F32)
            nc.scalar.mul(nmx[:], mx[:], -1.0)
            nmb_p = psT.tile([128, 1], F32, tag="tp", name="nmb_p")
            nc.tensor.matmul(nmb_p[:], lhsT=onesr[:], rhs=nmx[:],
                             start=True, stop=True)
            nmb = sb.tile([128, 1], F32)
            nc.vector.tensor_copy(nmb[:], nmb_p[:])
            ex = sb.tile([128, 16], F32)
            sums = sb.tile([128, 1], F32)
            nc.scalar.activation(ex[:], scf[:], ACT.Exp, bias=nmb[:],
                                 accum_out=sums[:])
            z_p = psT.tile([1, 1], F32, tag="tp", name="z_p")
            nc.tensor.matmul(z_p[:], lhsT=sums[:], rhs=ones,
                             start=True, stop=True)
            invz = sb.tile([1, 1], F32)
            nc.vector.reciprocal(invz[:], z_p[:])
            izb_p = psT.tile([128, 1], F32, tag="tp", name="izb_p")
            nc.tensor.matmul(izb_p[:], lhsT=onesr[:], rhs=invz[:],
                             start=True, stop=True)
            izb = sb.tile([128, 1], F32)
            nc.vector.tensor_copy(izb[:], izb_p[:])
            attn = sb.tile([128, 16], F32)
            nc.vector.tensor_scalar_mul(attn[:], ex[:], izb[:])
            nc.sync.dma_start(out=out_at_d[:].rearrange("(p f) -> p f", f=16),
                              in_=attn[:])
            el = sb.tile([128, 2], F32)
            nc.scalar.activation(el[:], sc_sb[:], ACT.Exp, bias=nmb[:])
            al = sb.tile([128, 2], BF16)
            nc.vector.tensor_scalar_mul(al[:], el[:], izb[:])

            # ---- P5: partial ctx over own encoder shard ------------------
            ctxp = psB.tile([128, HC], F32, tag="att")
            for j in range(HC):
                for t_i in range(2):
                    nc.tensor.matmul(ctxp[:, j:j + 1], lhsT=encN_l(t_i, j),
                                     rhs=al[:, t_i:t_i + 1],
                                     start=(t_i == 0), stop=(t_i == 1))
            ctx_sb = sb.tile([128, HC], F32)
            nc.vector.tensor_copy(ctx_sb[:], ctxp[:])
            ctx_b = dr.tile([H], F32)
            nc.scalar.dma_start(out=ctx_b[:].rearrange("(p f) -> p f", f=HC),
                                in_=ctx_sb[:])
            ctx_g = drs.tile([H], F32, addr_space="Shared", name="ctx_g")
            nc.gpsimd.collective_compute("AllReduce", ALU.add,
                                         replica_groups=rg,
                                         ins=[ctx_b[:]], outs=[ctx_g[:]])
            ctx8 = sb.tile([128, HC], F32)
            nc.scalar.dma_start(out=ctx8[:],
                                in_=ctx_g[:].rearrange("(p f) -> p f", f=HC))
            ctx8b = sb.tile([128, HC], BF16)
            nc.vector.tensor_copy(ctx8b[:], ctx8[:])

            # ---- P6: comb slice ------------------------------------------
            cbp = psB.tile([128, 1], F32, tag="att")
            for c in range(16):
                rhs = (ctx8b[:, c:c + 1] if c < HC
                       else hnew8b[:, c - HC:c - HC + 1])
                nc.tensor.matmul(cbp[:], lhsT=wcb_l(c), rhs=rhs,
                                 start=(c == 0), stop=(c == 15))
            cb = sb.tile([128, 1], F32)
            nc.scalar.activation(cb[:], cbp[:], ACT.Tanh, bias=bcb)
            cb_b = dr.tile([128], F32)
            nc.scalar.dma_start(out=cb_b[:].rearrange("(p f) -> p f", f=1),
                                in_=cb[:])
            cb_g = drs.tile([H], F32, addr_space="Shared", name="cb_g")
            nc.gpsimd.collective_compute("AllGather", ALU.bypass,
                                         replica_groups=rg,
                                         ins=[cb_b[:]], outs=[cb_g[:]])
            comb8 = sb.tile([128, HC], F32)
            nc.scalar.dma_start(out=comb8[:],
                                in_=cb_g[:].rearrange("(f p) -> p f", p=128))
            comb8b = sb.tile([128, HC], BF16)
            nc.vector.tensor_copy(comb8b[:], comb8[:])

            # ---- P7: W_out matvec + streaming stats ----------------------
            logits = sb.tile([128, NVT], F32)
            rmax = sb.tile([128, 1], F32)
            nc.vector.memset(rmax[:], NEG)
            nvb = BW // 128
            for vb in range(NB):
                ps = psW.tile([128, nvb], F32, tag="wops", name=f"wops_{vb}")
                for vt in range(nvb):
                    for c in range(HC):
                        nc.tensor.matmul(ps[:, vt:vt + 1],
                                         lhsT=wo_l(vb, c, vt),
                                         rhs=comb8b[:, c:c + 1],
                                         start=(c == 0), stop=(c == HC - 1))
                lsl = logits[:, vb * nvb:(vb + 1) * nvb]
                nc.vector.tensor_add(lsl, ps[:],
                                     bout_sb[:, vb * nvb:(vb + 1) * nvb])
                bm = sb.tile([128, 1], F32, tag="bm", name=f"bm_{vb}")
                nc.vector.tensor_reduce(bm[:], lsl, axis=AX, op=ALU.max)
                nc.vector.tensor_max(rmax[:], rmax[:], bm[:])

            # ---- P8: log-softmax stats + final ---------------------------
            rmt = psT.tile([1, 128], F32, tag="tp", name="rmt")
            nc.tensor.transpose(rmt[:], rmax[:], ident)
            mk = sb.tile([1, 1], F32)
            nc.vector.tensor_reduce(mk[:], rmt[:], axis=AX, op=ALU.max)
            nmk = sb.tile([1, 1], F32)
            nc.scalar.mul(nmk[:], mk[:], -1.0)
            nmkb_p = psT.tile([128, 1], F32, tag="tp", name="nmkb_p")
            nc.tensor.matmul(nmkb_p[:], lhsT=onesr[:], rhs=nmk[:],
                             start=True, stop=True)
            nmkb = sb.tile([128, 1], F32)
            nc.vector.tensor_copy(nmkb[:], nmkb_p[:])
            esc = sb.tile([128, NVT], F32)
            sumk = sb.tile([128, 1], F32)
            nc.scalar.activation(esc[:], logits[:], ACT.Exp, bias=nmkb[:],
                                 accum_out=sumk[:])
            sk_p = psT.tile([1, 1], F32, tag="tp", name="sk_p")
            nc.tensor.matmul(sk_p[:], lhsT=sumk[:], rhs=ones,
                             start=True, stop=True)
            st2 = sb.tile([1, 2], F32)
            nc.vector.tensor_copy(st2[0:1, 0:1], mk[:])
            nc.vector.tensor_copy(st2[0:1, 1:2], sk_p[:])
            st_b = dr.tile([2], F32)
            nc.scalar.dma_start(out=st_b[:].rearrange("(p f) -> p f", p=1),
                                in_=st2[:])
            st_g = drs.tile([2 * N_CORES], F32, addr_space="Shared",
                            name="st_g")
            nc.gpsimd.collective_compute("AllGather", ALU.bypass,
                                         replica_groups=rg,
                                         ins=[st_b[:]], outs=[st_g[:]])
            sg = sb.tile([1, 16], F32)
            nc.scalar.dma_start(out=sg[:],
                                in_=st_g[:].rearrange("(p f) -> p f", p=1))
            sgv = sg[:].rearrange("p (e two) -> p two e", two=2)
            mview = sgv[:, 0:1, :].rearrange("p a e -> p (a e)")
            sview = sgv[:, 1:2, :].rearrange("p a e -> p (a e)")
            gm = sb.tile([1, 1], F32)
            nc.vector.tensor_reduce(gm[:], mview, axis=AX, op=ALU.max)
            ngm = sb.tile([1, 1], F32)
            nc.scalar.mul(ngm[:], gm[:], -1.0)
            em8 = sb.tile([1, 8], F32)
            nc.scalar.activation(em8[:], mview, ACT.Exp, bias=ngm[:])
            zt8 = sb.tile([1, 8], F32)
            nc.vector.tensor_mul(zt8[:], em8[:], sview)
            zz = sb.tile([1, 1], F32)
            nc.vector.tensor_reduce(zz[:], zt8[:], axis=AX, op=ALU.add)
            lz = sb.tile([1, 1], F32)
            nc.scalar.activation(lz[:], zz[:], ACT.Ln)
            cc = sb.tile([1, 1], F32)
            nc.vector.tensor_add(cc[:], gm[:], lz[:])
            ncc = sb.tile([1, 1], F32)
            nc.scalar.mul(ncc[:], cc[:], -1.0)
            nccb_p = psT.tile([128, 1], F32, tag="tp", name="nccb_p")
            nc.tensor.matmul(nccb_p[:], lhsT=onesr[:], rhs=ncc[:],
                             start=True, stop=True)
            nccb = sb.tile([128, 1], F32)
            nc.vector.tensor_copy(nccb[:], nccb_p[:])
            outf = sb.tile([128, NVT], F32)
            nc.vector.tensor_scalar_add(outf[:], logits[:], nccb[:])
            nc.sync.dma_start(out=out_lg_d[:], in_=outf[:])

    nc.compile()
    _NC_CACHE["nc"] = nc
    return nc


def _prep_inputs(input_ids, hidden, encoder_outputs, emb_table, W_ih, W_hh,
                 b_ih, b_hh, W_comb, b_comb, W_out, b_out):
    """Shard + pre-layout all inputs per core (host-side, numpy)."""
    import ml_dtypes
    f = np.float32
    bf = ml_dtypes.bfloat16
    x_row = np.ascontiguousarray(emb_table[int(input_ids[0])], dtype=f)
    h_row = np.ascontiguousarray(hidden.reshape(H), dtype=f)
    x8 = np.ascontiguousarray(x_row.reshape(HC, 128).T)
    h8 = np.ascontiguousarray(h_row.reshape(HC, 128).T.astype(bf))
    onesr = np.ones((1, 128), dtype=f)
    b_ih = np.asarray(b_ih, dtype=f)
    b_hh = np.asarray(b_hh, dtype=f)
    bsum = b_ih + b_hh

    enc = np.asarray(encoder_outputs, dtype=f)
    encT_full = np.ascontiguousarray(enc.T)               # (H, S)
    W_ih = np.asarray(W_ih, dtype=f)
    W_hh = np.asarray(W_hh, dtype=f)
    W_comb = np.asarray(W_comb, dtype=f)
    W_out = np.asarray(W_out, dtype=f)
    b_out = np.asarray(b_out, dtype=f)
    b_comb = np.asarray(b_comb, dtype=f)

    in_maps = []
    for k in range(N_CORES):
        sl = slice(k * 128, (k + 1) * 128)
        # gate-sliced transposed GRU weights: (H, 384) cols = [r|z|n],
        # packed into quarters of 2 h-chunks: (4, 128, 2*384)
        wihT = np.empty((H, 384), dtype=bf)
        whhT = np.empty((H, 384), dtype=bf)
        for g in range(3):
            wihT[:, g * 128:(g + 1) * 128] = W_ih[g * H + k * 128:
                                                  g * H + (k + 1) * 128, :].T
            whhT[:, g * 128:(g + 1) * 128] = W_hh[g * H + k * 128:
                                                  g * H + (k + 1) * 128, :].T
        wihb = np.ascontiguousarray(
            wihT.reshape(4, 2, 128, 384).transpose(0, 2, 1, 3).reshape(
                4, 128, 768))
        whhb = np.ascontiguousarray(
            whhT.reshape(4, 2, 128, 384).transpose(0, 2, 1, 3).reshape(
                4, 128, 768))
        encT = encT_full[:, k * SPC:(k + 1) * SPC].astype(bf)  # (H, 256)
        encTb = np.ascontiguousarray(
            encT.reshape(4, 2, 128, 256).transpose(0, 2, 1, 3).reshape(
                4, 128, 512))
        encNb = np.ascontiguousarray(
            enc[k * SPC:(k + 1) * SPC, :].astype(bf).reshape(2, 128, H))
        wcbT = W_comb[sl, :].T.astype(bf)                  # (2H, 128)
        wcbb = np.ascontiguousarray(
            wcbT.reshape(4, 4, 128, 128).transpose(0, 2, 1, 3).reshape(
                4, 128, 512))
        brz = np.stack([bsum[0 * H + k * 128:0 * H + (k + 1) * 128],
                        bsum[1 * H + k * 128:1 * H + (k + 1) * 128]], axis=1)
        lo, hi = int(_OFFS[k]), int(_OFFS[k + 1])
        r = hi - lo
        woutT = np.zeros((H, VPC), dtype=bf)
        woutT[:, :r] = W_out[lo:hi, :].T.astype(bf)
        # (2*NB, 128, 4*BW): half-block i = (vb, h) covers chunks 4h..4h+3,
        # contiguous per partition row
        wob = np.ascontiguousarray(
            woutT.reshape(HC, 128, NB, BW)      # (c, p, vb, v)
            .transpose(2, 0, 1, 3)              # (vb, c, p, v)
            .reshape(NB, 2, 4, 128, BW)         # (vb, h, c4, p, v)
            .transpose(0, 1, 3, 2, 4)           # (vb, h, p, c4, v)
            .reshape(2 * NB, 128, 4 * BW))
        b_pad = np.full(VPC, NEG, dtype=f)
        b_pad[:r] = b_out[lo:hi]
        cpack = np.zeros((128, C_TOT), dtype=f)
        cpack[:, C_X8:C_X8 + 8] = x8
        cpack[:, C_HOWN] = h_row[sl]
        cpack[:, C_BRZ:C_BRZ + 2] = brz
        cpack[:, C_BIN] = b_ih[2 * H + k * 128:2 * H + (k + 1) * 128]
        cpack[:, C_BHN] = b_hh[2 * H + k * 128:2 * H + (k + 1) * 128]
        cpack[:, C_BCB] = b_comb[sl]
        cpack[:, C_ONE] = 1.0
        cpack[:, C_BOUT:C_BOUT + NVT] = b_pad.reshape(NVT, 128).T
        cpack[:, C_ID:C_ID + 128] = np.eye(128, dtype=f)
        in_maps.append({
            "cpack": cpack, "h8": h8, "onesr": onesr,
            "wihb": wihb, "whhb": whhb, "encTb": encTb, "encNb": encNb,
            "wcbb": wcbb, "wob": wob,
        })
    return in_maps


def _assemble(results):
    log_probs = np.empty((1, V), dtype=np.float32)
    for k in range(N_CORES):
        lg = np.asarray(results[k]["out_logits"]).reshape(128, NVT)
        shard = lg.T.reshape(VPC)
        lo, hi = int(_OFFS[k]), int(_OFFS[k + 1])
        log_probs[0, lo:hi] = shard[:hi - lo]
    h_new = np.asarray(results[0]["out_hnew"]).reshape(1, 1, H)
    attn = np.asarray(results[0]["out_attn"]).reshape(S)
    return log_probs, h_new, attn


_LAST_EXEC_NS = {"ns": None}


def kernel(**inputs):
    nc = _build_nc()
    in_maps = _prep_inputs(**inputs)
    if os.environ.get("KERNEL_SIM"):
        from concourse.bass_interp import MultiCoreSim
        sim = MultiCoreSim(nc, N_CORES)
        for i in range(N_CORES):
            for name, arr in in_maps[i].items():
                sim.cores[i].tensor(name)[:] = arr
        sim.simulate(check_with_hw=False)
        results = [{name: np.asarray(sim.cores[i].mem_tensor(name))
                    for name in ("out_logits", "out_hnew", "out_attn")}
                   for i in range(N_CORES)]
    else:
        trace = bool(os.environ.get("KERNEL_TRACE"))
        res = run_bass_kernel_spmd(nc, in_maps, list(range(N_CORES)),
                                   trace=trace)
        _LAST_EXEC_NS["ns"] = res.exec_time_ns
        results = res.results
    return _assemble(results)
